# revision 98
# baseline (speedup 1.0000x reference)
"""Trainium2 Bass kernel for nn_BSplineKAN_44719199486017.

2-layer B-spline KAN on x[4, 4096, 512]. Data-parallel over 8 NeuronCores:
core c handles batch b=c//2, sequence half h=c%2 (2048 rows).

Math: the 4 cubic Cox-de Boor basis functions reduce exactly to
    N_j(u) = sum_k C[j,k] * relu(k-u)^3,    u = 517*(x-min)/(max-min)
so the spline matmul becomes 4 plane matmuls with host-folded weights
    wk[f, o] = +sum_j C[j,k] * sw[o, 4f+j]
on planes r_k^3 with r_k = relu(k-u), built via a relu chain from
r_4 = relu(-su*x + (4+su*min)) (one fused scalar-engine activation),
squared (scalar/vector/pool mix) and cubed on the vector engine.

Per-layer min/max stats ride pair-group AllGathers on small (max, min)
tiles + a local fold — never a partner-data recompute. A dummy
AllGather at kernel start absorbs first-collective setup. Layer-0's
exchange is split per ft pair so ft0/ft1 spline matmuls start while
ft2/ft3's round-trip hides behind them; x tiles stream in halves so
the reduces begin on first touch. LayerNorm rsqrt runs entirely on the
DVE (fast-inverse-sqrt bit trick + two Newton steps, rel err ~4e-6),
so the scalar engine stays on the silu/relu act-table set for the
whole kernel (no act-table reloads anywhere). Exchange DMAs ride the
Activation HWDGE ring, away from the bulky weight/output descriptors
on the sync ring.

h1 is stored fp16 as t = 32*(h1 + 0.279): silu's flat minimum (-0.2785)
puts the spline-sensitive region near zero where fp16 is precise, and
min/max normalization is affine-invariant so u is unchanged. h1 rows
are transposed into the feature-major h1T tile by DMA-transpose
(XBAR), one descriptor per 128-row group — the PE never transposes and
PSUM keeps all 8 banks for matmul accumulation.

Matmul planes and weights are fp16 (fast weight load + 2x DVE),
accumulation stays fp32 in PSUM. LayerNorm+silu is fused into one
PSUM-read activation per group: silu(ps*rsig - mu*rsig). Plane building
runs one chunk ahead of the matmul/LN consumers on the vector queue so
the PE never waits on plane construction.
"""
import numpy as np
from contextlib import ExitStack

import concourse.bass as bass
import concourse.tile as tile
import concourse.mybir as mybir
from concourse import bacc
from concourse.bass_utils import run_bass_kernel_spmd

F32 = mybir.dt.float32
FP16 = mybir.dt.float16
I32 = mybir.dt.int32
AF = mybir.ActivationFunctionType
OP = mybir.AluOpType
AX = mybir.AxisListType

B, S, F = 4, 4096, 512
SH = S // 2          # rows per core
NFT = F // 128       # feature tiles (4)
PCW = 512            # plane-chunk width (rows per plane build)
NPC = SH // PCW      # plane chunks (4)
GPP = PCW // 128     # row groups per chunk (4)
N_CORES = 8
KNOT_SCALE = 517.0
EPS = 1e-5
HSHIFT = 0.279        # just below silu's minimum (-0.27846)
HSCALE = 32.0         # h1 stored fp16 as t = HSCALE*(h1 + HSHIFT)
PAIR_GROUPS = [[0, 1], [2, 3], [4, 5], [6, 7]]
FMAX = 3.0e38

BASIS_C = np.array([
    [1.0, 0.0, 0.0, 0.0],
    [-2.0, 0.25, 0.0, 0.0],
    [1.5, -0.75, 1.0 / 6.0, 0.0],
    [-2.0 / 3.0, 1.0, -2.0 / 3.0, 1.0 / 6.0],
], dtype=np.float64)  # [j, k-1]

_CACHE = {}


def _build(sim=False, fast_gb=True):
    nc = bacc.Bacc("TRN2", target_bir_lowering=False, debug=False,
                   num_devices=1 if sim else N_CORES)
    nc._sim_mode = sim

    xT = nc.declare_dram_parameter("xT", [F, SH], F32, isOutput=False)
    W0 = nc.declare_dram_parameter("W0", [F, 5, F], FP16, isOutput=False)
    W1 = nc.declare_dram_parameter("W1", [F, 5, F], FP16, isOutput=False)
    GB0 = nc.declare_dram_parameter("GB0", [128, 2, F], F32, isOutput=False)
    GB1 = nc.declare_dram_parameter("GB1", [128, 2, F], F32, isOutput=False)
    OUT = nc.declare_dram_parameter("out", [SH, F], F32, isOutput=True)

    with ExitStack() as ctx:
        tc = ctx.enter_context(tile.TileContext(nc))
        dram = ctx.enter_context(tc.tile_pool(name="dram", bufs=1, space="DRAM"))
        wpool = ctx.enter_context(tc.tile_pool(name="w", bufs=1))
        xpool = ctx.enter_context(tc.tile_pool(name="x", bufs=1))
        hpool = ctx.enter_context(tc.tile_pool(name="h", bufs=1))
        lpool = ctx.enter_context(tc.tile_pool(name="l", bufs=2))
        stat = ctx.enter_context(tc.tile_pool(name="st", bufs=1))
        rpool = ctx.enter_context(tc.tile_pool(name="r", bufs=2))
        psum = ctx.enter_context(tc.tile_pool(name="ps", bufs=8, space="PSUM"))

        # ---- input loads, all on the sync queue in priority order -------
        # x -> W0 base slices (gate base matmuls) -> W0 spline -> W1
        xts = [xpool.tile([128, SH], F32, tag=f"x{ft}", name=f"x{ft}")
               for ft in range(NFT)]
        for hh in range(2):
            hs = slice(hh * (SH // 2), (hh + 1) * (SH // 2))
            for ft in range(NFT):
                src_ = xT.rearrange("(ft p) s -> ft p s", p=128)[ft]
                nc.sync.dma_start(xts[ft][:, hs], src_[:, hs])
        # collective warmup: first collective in a NEFF pays extra setup;
        # burn it early on a dummy buffer while the x loads stream
        if not getattr(nc, "_sim_mode", False):
            wu_in = dram.tile([128, 8], F32, tag="wu_in", name="wu_in")
            wu_out = dram.tile([2, 128, 8], F32, tag="wu_out", name="wu_out")
            nc.gpsimd.collective_compute(
                "AllGather", OP.bypass,
                ins=[wu_in.opt()], outs=[wu_out.opt()],
                replica_groups=PAIR_GROUPS,
            )

        wts = [None, None]
        wl0 = []
        for ft in range(NFT):
            t = wpool.tile([128, 5, F], FP16, tag=f"w0{ft}", name=f"w0{ft}")
            src_ = W0.rearrange("(ft p) c n -> ft p c n", p=128)[ft]
            nc.sync.dma_start(t[:, 0:1, :], src_[:, 0:1, :])
            wl0.append(t)
        wts[0] = wl0
        for ft in range(NFT):
            nc.sync.dma_start(
                wl0[ft][:, 1:5, :],
                W0.rearrange("(ft p) c n -> ft p c n", p=128)[ft][:, 1:5, :])

        wl1 = []
        for ft in range(NFT):
            t = wpool.tile([128, 5, F], FP16, tag=f"w1{ft}", name=f"w1{ft}")
            nc.sync.dma_start(
                t[:], W1.rearrange("(ft p) c n -> ft p c n", p=128)[ft])
            wl1.append(t)
        wts[1] = wl1

        gbts = []
        if not fast_gb:
            for li, GBp in enumerate((GB0, GB1)):
                t = wpool.tile([128, 2, F], F32, tag=f"gb{li}", name=f"gb{li}")
                nc.sync.dma_start(t[:], GBp[:])
                gbts.append(t)

        def fold_mm(dst, a, b):
            """dst/a/b [128, 8] as (max 0:4, min 4:8)."""
            nc.vector.tensor_tensor(dst[:, 0:4], a[:, 0:4], b[:, 0:4],
                                    op=OP.max)
            nc.vector.tensor_tensor(dst[:, 4:8], a[:, 4:8], b[:, 4:8],
                                    op=OP.min)

        # ---- layer-0 own stats on the vector queue ----------------------
        # reduced per half-tile (halves DMA'd separately) so the vector
        # engine starts as soon as the first half lands.
        # pk layout: cols 0:4 = per-ft max, cols 4:8 = per-ft min
        pkh = stat.tile([128, 16], F32, tag="pkh", name="pkh")
        for ft in range(NFT):
            for hh in range(2):
                hs = slice(hh * (SH // 2), (hh + 1) * (SH // 2))
                nc.vector.tensor_reduce(pkh[:, hh * 8 + ft:hh * 8 + ft + 1],
                                        xts[ft][:, hs], axis=AX.X, op=OP.max)
                nc.vector.tensor_reduce(
                    pkh[:, hh * 8 + 4 + ft:hh * 8 + 5 + ft],
                    xts[ft][:, hs], axis=AX.X, op=OP.min)


        def suchain(st_, layer):
            """st_ [128, 8] (max, min) -> (nsu, fb) [128, 4].

            Shallow form: nsu = -K*rcp; fb = K*(min*rcp) + 4 — dependency
            depth 4 instead of 6 (this chain trails each stats exchange)."""
            rng = stat.tile([128, 4], F32, tag=f"rng{layer}", name=f"rng{layer}")
            nc.vector.tensor_tensor(rng[:], st_[:, 0:4], st_[:, 4:8],
                                    op=OP.subtract)
            rcp = stat.tile([128, 4], F32, tag=f"rcp{layer}", name=f"rcp{layer}")
            nc.vector.reciprocal(rcp[:], rng[:])
            nsu = stat.tile([128, 4], F32, tag=f"nsu{layer}", name=f"nsu{layer}")
            nc.vector.tensor_scalar(nsu[:], rcp[:], -KNOT_SCALE, None,
                                    op0=OP.mult)
            sb = stat.tile([128, 4], F32, tag=f"sb{layer}", name=f"sb{layer}")
            nc.vector.tensor_tensor(sb[:], st_[:, 4:8], rcp[:], op=OP.mult)
            fb = stat.tile([128, 4], F32, tag=f"fb{layer}", name=f"fb{layer}")
            nc.vector.tensor_scalar(fb[:], sb[:], KNOT_SCALE, 4.0,
                                    op0=OP.mult, op1=OP.add)
            return nsu, fb

        def exchange(src, tagp, q=None):
            """Pair AllGather of [128, 8] (max, min) stats + local fold.

            The tiny in/out DMAs ride the Activation HWDGE ring by default —
            the sync ring has multi-us weight/output descriptors queued."""
            q = q or nc.scalar
            cc_in = dram.tile([128, 8], F32, tag=f"{tagp}_in",
                              name=f"{tagp}_in")
            cc_out = dram.tile([2, 128, 8], F32, tag=f"{tagp}_out",
                               name=f"{tagp}_out")
            q.dma_start(cc_in[:], src[:])
            if getattr(nc, "_sim_mode", False):
                for r_ in range(2):
                    q.dma_start(cc_out[:][r_], cc_in[:])
            else:
                nc.gpsimd.collective_compute(
                    "AllGather", OP.bypass,
                    ins=[cc_in.opt()], outs=[cc_out.opt()],
                    replica_groups=PAIR_GROUPS,
                )
            res2 = stat.tile([128, 2, 8], F32, tag=f"{tagp}_r2",
                             name=f"{tagp}_r2")
            q.dma_start(res2[:],
                        cc_out[:].rearrange("r p c -> p r c"))
            res = stat.tile([128, 8], F32, tag=f"{tagp}_r", name=f"{tagp}_r")
            fold_mm(res, res2[:, 0, :], res2[:, 1, :])
            return res

        # layer-0 exchange split per ft pair: ft0/ft1 stats fire as soon as
        # those tiles are reduced, so their spline planes (and matmuls)
        # start while ft2/ft3's reduce + exchange latency hides behind them
        nsu0 = stat.tile([128, 4], F32, tag="nsu0", name="nsu0")
        fb0 = stat.tile([128, 4], F32, tag="fb0", name="fb0")
        for ph in range(2):
            f0 = 2 * ph
            cs = slice(f0, f0 + 2)
            with tc.high_priority():
                pk2 = stat.tile([128, 4], F32, tag=f"pk2_{ph}",
                                name=f"pk2_{ph}")
                nc.vector.tensor_tensor(pk2[:, 0:2], pkh[:, f0:f0 + 2],
                                        pkh[:, 8 + f0:10 + f0], op=OP.max)
                nc.vector.tensor_tensor(pk2[:, 2:4], pkh[:, 4 + f0:6 + f0],
                                        pkh[:, 12 + f0:14 + f0], op=OP.min)
                cc_in = dram.tile([128, 4], F32, tag=f"cp{ph}_in",
                                  name=f"cp{ph}_in")
                cc_out = dram.tile([2, 128, 4], F32, tag=f"cp{ph}_out",
                                   name=f"cp{ph}_out")
                nc.scalar.dma_start(cc_in[:], pk2[:])
                if getattr(nc, "_sim_mode", False):
                    for r_ in range(2):
                        nc.scalar.dma_start(cc_out[:][r_], cc_in[:])
                else:
                    nc.gpsimd.collective_compute(
                        "AllGather", OP.bypass,
                        ins=[cc_in.opt()], outs=[cc_out.opt()],
                        replica_groups=PAIR_GROUPS,
                    )
                res2 = stat.tile([128, 2, 4], F32, tag=f"cp{ph}_r2",
                                 name=f"cp{ph}_r2")
                nc.scalar.dma_start(res2[:],
                                    cc_out[:].rearrange("r p c -> p r c"))
                res = stat.tile([128, 4], F32, tag=f"cp{ph}_r",
                                name=f"cp{ph}_r")
                nc.vector.tensor_tensor(res[:, 0:2], res2[:, 0, 0:2],
                                        res2[:, 1, 0:2], op=OP.max)
                nc.vector.tensor_tensor(res[:, 2:4], res2[:, 0, 2:4],
                                        res2[:, 1, 2:4], op=OP.min)
                # su chain on the 2-wide pair
                rng = stat.tile([128, 2], F32, tag=f"rng0{ph}",
                                name=f"rng0{ph}")
                nc.vector.tensor_tensor(rng[:], res[:, 0:2], res[:, 2:4],
                                        op=OP.subtract)
                rcp = stat.tile([128, 2], F32, tag=f"rcp0{ph}",
                                name=f"rcp0{ph}")
                nc.vector.reciprocal(rcp[:], rng[:])
                nc.vector.tensor_scalar(nsu0[:, cs], rcp[:], -KNOT_SCALE,
                                        None, op0=OP.mult)
                sb = stat.tile([128, 2], F32, tag=f"sb0{ph}",
                               name=f"sb0{ph}")
                nc.vector.tensor_tensor(sb[:], res[:, 2:4], rcp[:],
                                        op=OP.mult)
                nc.vector.tensor_scalar(fb0[:, cs], sb[:], KNOT_SCALE, 4.0,
                                        op0=OP.mult, op1=OP.add)

        # h1T_all[p, ft, s] = h1 feature (128*ft+p) at row s (fp16, shifted)
        h1T = hpool.tile([128, NFT, SH], FP16, tag="h1T", name="h1T")
        csts = {}
        for cname, cval in (("nsh", -HSHIFT), ("ish", 1.0 / HSCALE)):
            t = stat.tile([128, 1], F32, tag=f"c{cname}", name=f"c{cname}")
            nc.gpsimd.memset(t[:], cval)
            csts[cname] = t
        magic = stat.tile([128, GPP], I32, tag="magic", name="magic")
        nc.gpsimd.memset(magic[:], 0x5f3759df)
        pk1 = stat.tile([128, 16], F32, tag="pk1", name="pk1")
        pk1q = stat.tile([128, 32], F32, tag="pk1q", name="pk1q")

        # base planes silu(src), fp16. Layer 0 per-chunk (so base matmuls
        # start as soon as the chunk-0 columns exist); layer 1 incrementally.
        sils = [[None] * NFT for _ in range(2)]
        for ft in range(NFT):
            sils[0][ft] = lpool.tile([128, SH], FP16, tag=f"sil0_{ft}",
                                     name=f"sil0_{ft}", bufs=1)
            sils[1][ft] = lpool.tile([128, SH], FP16, tag=f"sil1_{ft}",
                                     name=f"sil1_{ft}", bufs=1)
        def fill_sil0(pcs):
            for pc in pcs:
                for ft in range(NFT):
                    nc.scalar.activation(
                        sils[0][ft][:, pc * PCW:(pc + 1) * PCW],
                        xts[ft][:, pc * PCW:(pc + 1) * PCW], AF.Silu)

        # chunks 0/1 feed the pre-opened base groups now; chunks 2/3 are
        # emitted after the first plane build so the scalar queue never
        # blocks the chunk-0 r4s behind them
        fill_sil0((0, 1))

        pss = {}

        def open_group(li, g):
            ps = psum.tile([128, F], F32, tag="y", name="y")
            pss[(li, g)] = ps
            for ft in range(NFT):
                nc.tensor.matmul(ps[:],
                                 sils[li][ft][:, g * 128:(g + 1) * 128],
                                 wts[li][ft][:, 0, :],
                                 start=(ft == 0), stop=False)

        for g in range(8):
            open_group(0, g)

        stats = [(nsu0, fb0), None]

        def do_transposes(g, rowt):
            nc.sync.dma_start_transpose(
                h1T[:, :, g * 128:(g + 1) * 128], rowt[:])
            if g >= 12:
                # last-chunk stats per group so the pair exchange can fire
                # right after the final transpose
                q = g - 12
                for ft in range(NFT):
                    nc.vector.tensor_reduce(
                        pk1q[:, q * 8 + ft:q * 8 + ft + 1],
                        h1T[:, ft, g * 128:(g + 1) * 128], axis=AX.X,
                        op=OP.max)
                    nc.vector.tensor_reduce(
                        pk1q[:, q * 8 + 4 + ft:q * 8 + 5 + ft],
                        h1T[:, ft, g * 128:(g + 1) * 128], axis=AX.X,
                        op=OP.min)

        def build_planes(li, pc):
            """Enqueue spline-plane construction for chunk pc of layer li."""
            nsu, fb = stats[li]
            psl = slice(pc * PCW, (pc + 1) * PCW)
            planes = [[None] * 5 for _ in range(NFT)]
            for ft in range(NFT):
                src = xts[ft][:, psl] if li == 0 else h1T[:, ft, psl]
                r4 = lpool.tile([128, PCW], FP16, tag="r4", name="r4",
                                bufs=4)
                nc.scalar.activation(r4[:], src, AF.Relu,
                                     bias=fb[:, ft:ft + 1],
                                     scale=nsu[:, ft:ft + 1])
                rks = {4: r4}
                for k in (3, 2, 1):
                    rk = lpool.tile([128, PCW], FP16, tag="rk", name="rk",
                                    bufs=6)
                    nc.vector.tensor_scalar(rk[:], r4[:], float(4 - k),
                                            0.0, op0=OP.subtract,
                                            op1=OP.max)
                    rks[k] = rk
                for k in (4, 3, 2, 1):
                    rk = rks[k]
                    qk = lpool.tile([128, PCW], FP16, tag="qk", name="qk",
                                    bufs=5)
                    if k == 4:
                        nc.scalar.activation(qk[:], rk[:], AF.Square)
                    elif k == 3:
                        nc.vector.tensor_tensor(qk[:], rk[:], rk[:],
                                                op=OP.mult)
                    else:
                        nc.gpsimd.tensor_tensor(qk[:], rk[:], rk[:],
                                                op=OP.mult)
                    lk = lpool.tile([128, PCW], FP16, tag=f"lk{k}_{ft}",
                                    name=f"lk{k}_{ft}", bufs=3)
                    nc.vector.tensor_tensor(lk[:], qk[:], rk[:],
                                            op=OP.mult)
                    planes[ft][k] = lk
            return planes

        def emit_group(li, pc, gg, gps, rsp, nmr, stp):
            """LN+silu epilogue for one 128-row group, then transpose/store."""
            g = pc * GPP + gg
            ps = gps[gg]
            if fast_gb:
                row = rpool.tile([128, F], F32, tag="row", name="row",
                                 bufs=3)
                nc.scalar.activation(row[:], ps[:], AF.Silu,
                                     bias=nmr[:, gg:gg + 1],
                                     scale=rsp[:, gg:gg + 1])
            else:
                t2 = rpool.tile([128, F], F32, tag="t2", name="t2", bufs=2)
                nc.vector.tensor_scalar(t2[:], ps[:], stp[:, 0, gg:gg + 1],
                                        rsp[:, gg:gg + 1],
                                        op0=OP.subtract, op1=OP.mult)
                nc.vector.tensor_tensor(t2[:], t2[:], gbts[li][:, 0, :],
                                        op=OP.mult)
                nc.gpsimd.tensor_tensor(t2[:], t2[:], gbts[li][:, 1, :],
                                        op=OP.add)
                row = rpool.tile([128, F], F32, tag="row", name="row",
                                 bufs=3)
                nc.scalar.activation(row[:], t2[:], AF.Silu)

            if li == 0:
                rowt = rpool.tile([128, F], FP16, tag="rowt", name="rowt",
                                  bufs=2)
                nc.scalar.activation(rowt[:], row[:], AF.Copy,
                                     bias=HSCALE * HSHIFT, scale=HSCALE)
                do_transposes(g, rowt)
            else:
                nc.sync.dma_start(OUT[:][g * 128:(g + 1) * 128, :], row[:])

        for li in range(2):
            wt = wts[li]
            planes_next = None

            for pc in range(NPC):
                planes = planes_next or build_planes(li, pc)
                planes_next = None
                if li == 0 and pc == 0:
                    fill_sil0((2, 3))
                # h1 stats for completed columns, emitted one chunk after
                # their transposes land so they slot into real queue holes
                # without head-blocking and are done before the layer tail
                if li == 0 and pc in (2, 3):
                    h0 = 0 if pc == 2 else 1024
                    hw_ = 1024 if pc == 2 else 512
                    c0 = 0 if pc == 2 else 8
                    for ft in range(NFT):
                        nc.vector.tensor_reduce(
                            pk1[:, c0 + ft:c0 + ft + 1],
                            h1T[:, ft, h0:h0 + hw_], axis=AX.X, op=OP.max)
                        nc.vector.tensor_reduce(
                            pk1[:, c0 + 4 + ft:c0 + 5 + ft],
                            h1T[:, ft, h0:h0 + hw_], axis=AX.X, op=OP.min)
                        if pc == 2:
                            # sil fill for the pre-opened layer-1 base
                            # groups; cols 0-1023 are final after chunk 1
                            nc.scalar.activation(
                                sils[1][ft][:, 0:1024], h1T[:, ft, 0:1024],
                                AF.Silu, bias=csts["nsh"][:],
                                scale=csts["ish"][:])

                # ---- row-group matmuls for this chunk --------------------
                stp = stat.tile([128, 2, GPP], F32, tag="stp", name="stp",
                                bufs=3)
                gps = []
                for gg in range(GPP):
                    g = pc * GPP + gg
                    if (li, g) not in pss:
                        open_group(li, g)
                    gps.append(pss.pop((li, g)))
                # chunk 0 of layer 0: ft0/ft1 spline matmuls for ALL groups
                # first (their stats exchange lands earlier), then ft2/ft3
                if li == 0 and pc == 0:
                    ft_passes = [(0, 1), (2, 3)]
                else:
                    ft_passes = [tuple(range(NFT))]
                for pi, fts_ in enumerate(ft_passes):
                    last_pass = pi == len(ft_passes) - 1
                    for gg in range(GPP):
                        ps = gps[gg]
                        for ft in fts_:
                            for k in (4, 3, 2, 1):
                                stop = (last_pass and ft == fts_[-1]
                                        and k == 1)
                                nc.tensor.matmul(
                                    ps[:],
                                    planes[ft][k][:, gg * 128:(gg + 1) * 128],
                                    wt[ft][:, k, :],
                                    start=False, stop=stop)

                # prebuild the next chunk's planes ahead of this chunk's
                # LN/stat work on the vector queue so the PE never waits on
                # plane construction at chunk boundaries
                if pc + 1 < NPC:
                    planes_next = build_planes(li, pc + 1)

                # ---- batched LN + silu epilogue --------------------------
                # (last chunk of each layer: per-group so the layer tail —
                # the stats exchange for L0, the output drain for L1 —
                # starts right after the final group's matmuls)
                halves = 4 if pc == NPC - 1 else 1
                rsp = stat.tile([128, GPP], F32, tag="rsp", name="rsp",
                                bufs=2)
                nmr = stat.tile([128, GPP], F32, tag="nmr", name="nmr",
                                bufs=2)
                for hb in range(halves):
                    glo = hb * (GPP // halves)
                    ghi = (hb + 1) * (GPP // halves)
                    gsl = slice(glo, ghi)
                    for gg in range(glo, ghi):
                        st6 = stat.tile([128, 6], F32, tag="st6", name="st6",
                                        bufs=4)
                        nc.vector.bn_stats(st6[:], gps[gg][:])
                        nc.vector.bn_aggr(stp[:, :, gg], st6[:])
                    # rsqrt on DVE via fast-inverse-sqrt + 2 Newton steps —
                    # keeps the scalar engine on one act-table set (no sqrt
                    # table loads). rel err ~4e-6, plenty under fp16 noise.
                    gw = ghi - glo
                    vep = stat.tile([128, GPP], F32, tag="vep", name="vep",
                                    bufs=2)
                    nc.vector.tensor_scalar(vep[:, gsl], stp[:, 1, gsl], EPS,
                                            None, op0=OP.add)
                    vh = stat.tile([128, GPP], F32, tag="vh", name="vh",
                                   bufs=2)
                    nc.vector.tensor_scalar(vh[:, gsl], vep[:, gsl], 0.5,
                                            None, op0=OP.mult)
                    sdi = stat.tile([128, GPP], I32, tag="sdi", name="sdi",
                                    bufs=2)
                    nc.vector.tensor_scalar(sdi[:, gsl],
                                            vep[:, gsl].bitcast(I32), 1,
                                            None, op0=OP.arith_shift_right)
                    nc.vector.tensor_tensor(sdi[:, gsl], magic[:, 0:gw],
                                            sdi[:, gsl], op=OP.subtract)
                    yy = sdi[:, gsl].bitcast(F32)
                    nt1 = stat.tile([128, GPP], F32, tag="nt1", name="nt1",
                                    bufs=2)
                    # L0's last chunk sits on the layer-transition critical
                    # chain: one Newton step (rsig rel err ~0.2%, harmless
                    # in the shifted-fp16 h1 representation) instead of two
                    n_it = 2
                    for it in range(n_it):
                        nc.vector.tensor_tensor(nt1[:, gsl], yy, yy,
                                                op=OP.mult)
                        nc.vector.tensor_tensor(nt1[:, gsl], nt1[:, gsl],
                                                vh[:, gsl], op=OP.mult)
                        nc.vector.tensor_scalar(nt1[:, gsl], nt1[:, gsl],
                                                -1.0, 1.5, op0=OP.mult,
                                                op1=OP.add)
                        dst = (rsp[:, gsl] if it == n_it - 1 else sdi[:, gsl]
                               .bitcast(F32))
                        nc.vector.tensor_tensor(dst, yy, nt1[:, gsl],
                                                op=OP.mult)
                        yy = dst
                    nc.vector.scalar_tensor_tensor(nmr[:, gsl],
                                                   stp[:, 0, gsl], -1.0,
                                                   rsp[:, gsl], op0=OP.mult,
                                                   op1=OP.mult)

                    for gg in range(glo, ghi):
                        emit_group(li, pc, gg, gps, rsp, nmr, stp)

                # sil fills for layer-1 chunks 2/3 happen during layer-1
                # where the scalar engine has slack
                if li == 1 and pc in (0, 1):
                    h0 = 1024 + pc * 512
                    for ft in range(NFT):
                        nc.scalar.activation(sils[1][ft][:, h0:h0 + 512],
                                             h1T[:, ft, h0:h0 + 512],
                                             AF.Silu, bias=csts["nsh"][:],
                                             scale=csts["ish"][:])

            if li == 0:
                # layer-1 base matmuls don't need stats — open them so the
                # PE chews on them while the exchange round-trips (8 run
                # immediately; 4 more as layer-0 frees PSUM banks)
                for g in range(8):
                    open_group(1, g)
                with tc.high_priority():
                    st1a = stat.tile([128, 8], F32, tag="st1a", name="st1a")
                    nc.vector.tensor_tensor(st1a[:, 0:4], pk1[:, 0:4],
                                            pk1[:, 8:12], op=OP.max)
                    nc.vector.tensor_tensor(st1a[:, 4:8], pk1[:, 4:8],
                                            pk1[:, 12:16], op=OP.min)
                    q01 = stat.tile([128, 8], F32, tag="q01", name="q01")
                    fold_mm(q01, pk1q[:, 0:8], pk1q[:, 8:16])
                    q23 = stat.tile([128, 8], F32, tag="q23", name="q23")
                    fold_mm(q23, pk1q[:, 16:24], pk1q[:, 24:32])
                    qq = stat.tile([128, 8], F32, tag="qq", name="qq")
                    fold_mm(qq, q01, q23)
                    st1 = stat.tile([128, 8], F32, tag="st1", name="st1")
                    fold_mm(st1, st1a, qq)
                    res = exchange(st1, "cc1")
                    stats[1] = suchain(res, 1)

    nc.compile()
    return nc


def _prep_inputs(x, bw0, sw0, g0, b0, bw1, sw1, g1, b1):
    def fold(bw, sw):
        sw4 = np.asarray(sw, np.float64).reshape(F, F, 4)
        wk = np.einsum('ofj,jk->kfo', sw4, BASIS_C)           # [4, f_in, o]
        W = np.empty((F, 5, F), np.float32)
        W[:, 0, :] = np.asarray(bw, np.float32).T
        for k in range(4):
            W[:, k + 1, :] = wk[k].astype(np.float32)
        return W.astype(np.float16)

    def gbpack(g, b):
        GB = np.empty((128, 2, F), np.float32)
        GB[:, 0, :] = np.asarray(g, np.float32)[None, :]
        GB[:, 1, :] = np.asarray(b, np.float32)[None, :]
        return GB

    W0 = fold(bw0, sw0)
    W1 = fold(bw1, sw1)
    GB0 = gbpack(g0, b0)
    GB1 = gbpack(g1, b1)

    xs = []
    for c in range(N_CORES):
        b_, h_ = divmod(c, 2)
        xs.append(np.ascontiguousarray(
            np.asarray(x, np.float32)[b_, h_ * SH:(h_ + 1) * SH, :].T))

    in_maps = []
    for c in range(N_CORES):
        in_maps.append(dict(xT=xs[c], W0=W0, W1=W1, GB0=GB0, GB1=GB1))
    return in_maps


def kernel(x, bw0, sw0, g0, b0, bw1, sw1, g1, b1):
    fast = (np.all(np.asarray(g0) == 1) and np.all(np.asarray(g1) == 1)
            and np.all(np.asarray(b0) == 0) and np.all(np.asarray(b1) == 0))
    key = "nc_fast" if fast else "nc_gen"
    if key not in _CACHE:
        _CACHE[key] = _build(fast_gb=fast)
    nc = _CACHE[key]
    in_maps = _prep_inputs(x, bw0, sw0, g0, b0, bw1, sw1, g1, b1)
    res = run_bass_kernel_spmd(nc, in_maps, list(range(N_CORES)))
    out = np.empty((B, S, F), np.float32)
    for c in range(N_CORES):
        b_, h_ = divmod(c, 2)
        out[b_, h_ * SH:(h_ + 1) * SH, :] = res.results[c]["out"]
    return out
